# revision 21
# baseline (speedup 1.0000x reference)
"""Trainium2 Bass kernel for the KSubspaceBaseModel objective.

Reference computes, for B=2048 samples x (B, D=1024) and subspace bases
Us (R=4, K=16, D, d=32):
    z = x @ U; x_ = z @ U^T; loss = 0.5*||x - x_||^2  (per b, r, k)
    obj_r = mean_b min_k loss

Algebraic collapse: with G = U^T U and L = chol(I - 0.5 G) folded into
U host-side (Ut = U @ L),
    loss = 0.5||x||^2 - ||Ut^T x||^2
so the device only computes z~ = Ut^T x, squares it, sums each subspace's
32 latent columns, and takes max_k.  obj_r = 0.5*mean||x||^2 - mean_b max_k,
with the 0.5*mean||x||^2 scalar folded in on the host.

Inputs are quantized to fp8e4 (x at scale 1, Ut at scale 256) and the
matmuls run in DoubleRow perf mode (2 fp8 MACs/cell, contraction 256 per
matmul): 32 matmuls of [128,2,128]x[128,2,512] per core instead of 64
bf16 ones, and half the DMA bytes.  The fp8 error only perturbs the
~0.4-magnitude correction term of a ~511 objective.

Sharding over 8 cores: 4 batch quarters (512 samples) x 2 subspace halves
(32 subspaces = 2 whole replicates), so the k-max is core-local.

Schedule: a short fp8 warmup bridges the Tile preamble end (~7.2us) to
first-data (~9.5us) so the HAM clock-gate ramp (a ~3.4-6.8us *time*
window at half PE clock) burns off the critical path.  The real stream
is group-sequential (nh-outer, then batch chunk, contraction innermost)
so each PSUM group's square/reduce/max epilogue overlaps later groups'
matmuls; only the last group's chain trails the stream.
"""

import numpy as np
import ml_dtypes

import concourse.bass as bass
import concourse.bacc as bacc
import concourse.mybir as mybir
import concourse.tile as tile
from concourse.bass_utils import run_bass_kernel_spmd

B, D, R, K, d = 2048, 1024, 4, 16, 32
NCORES = 8
NB = B // 4          # 512 samples per core
KC = D // 128        # 8 contraction chunks of 128
BC = NB // 128       # 4 batch chunks per core
USCALE = 256.0       # Ut is quantized at x256; z~^2 lands x65536
FP8 = mybir.dt.float8e4
FP32 = mybir.dt.float32
BF16 = mybir.dt.bfloat16
DR = mybir.MatmulPerfMode.DoubleRow

_COMPILED = {}
LAST_RESULTS = None

N_WARM = 17          # bridge preamble-end -> first-data on the PE


def _build():
    nc = bacc.Bacc("TRN2", target_bir_lowering=False, debug=False)
    # host pre-arranges so each partition's DMA read is one contiguous run
    xt = nc.dram_tensor("xt", [128, KC * NB], FP8, kind="ExternalInput")
    u0 = nc.dram_tensor("u0", [128, KC * 512], FP8, kind="ExternalInput")
    u1 = nc.dram_tensor("u1", [128, KC * 512], FP8, kind="ExternalInput")
    outp = nc.dram_tensor("outp", [128, 2 * BC], FP32, kind="ExternalOutput")

    # x is batch-chunk-major host-side: [128, bc, kc, c] so one 128KB DMA
    # delivers everything a (bc, *) matmul group needs from x
    xt_v = xt.ap().rearrange("p (b o c) -> p b o c", b=BC, o=KC)
    u_v = [u.ap().rearrange("p (o n) -> p o n", o=KC) for u in (u0, u1)]

    with tile.TileContext(nc) as tc:
        with (
            tc.tile_pool(name="xsb", bufs=1) as xpool,
            tc.tile_pool(name="usb", bufs=1) as upool,
            tc.tile_pool(name="esb", bufs=3) as epool,
            tc.tile_pool(name="asb", bufs=2) as apool,
            tc.tile_pool(name="single", bufs=1) as spool,
            tc.tile_pool(name="zp", bufs=1, space="PSUM") as zpool,
        ):
            # Two HWDGE rings, ~630ns fixed cost per trigger, ordered by
            # first use: scalar streams u0 in kc-quarters (paces the first
            # group's cold matmuls), sync streams the per-group x chunks
            # then u1, which isn't needed until the nh=1 half-stream.
            x_sb = xpool.tile([128, BC, KC, 128], FP8, tag="x", name="x")
            u_sb = [upool.tile([128, KC, 512], FP8, tag=f"u{nh}",
                               name=f"u{nh}") for nh in range(2)]
            for q in range(4):
                nc.scalar.dma_start(u_sb[0][:, 2 * q:2 * q + 2, :],
                                    u_v[0][:, 2 * q:2 * q + 2, :])
            for bc in range(BC):
                nc.sync.dma_start(x_sb[:, bc], xt_v[:, bc])
            nc.sync.dma_start(u_sb[1][:, 0:4, :], u_v[1][:, 0:4, :])
            nc.sync.dma_start(u_sb[1][:, 4:8, :], u_v[1][:, 4:8, :])

            ostage = spool.tile([128, 2 * BC], FP32, tag="ostage")

            # PE warm-up: dep-free fp8 matmuls (FD=256, ~250ns cold) on a
            # memset tile keep the PE busy from preamble-end so the HAM
            # half-clock ramp (a gap-free ~3.4-6.8us activity window)
            # overlaps the input DMA wait.  Slight overshoot past data
            # arrival is preferable to an idle gap, which delays the
            # un-throttle by a full window.  Shares a PSUM bank with the
            # last-emitted real group, which starts late enough not to
            # collide.
            warm = spool.tile([128, 256], FP8, tag="warm")
            nc.vector.memset(warm[:], 0.0)
            wp = zpool.tile([128, 512], FP32, tag="zp_3_1", name="warm_ps")

            def filler(n):
                # dep-free ~107ns matmuls: keep the PE busy where the
                # stream would otherwise stall on DMA, so the HAM activity
                # window never sees a gap
                for _ in range(n):
                    nc.tensor.matmul(wp[:, 0:128], warm[:, 0:128],
                                     warm[:, 128:256],
                                     start=True, stop=True,
                                     skip_group_check=True)

            filler(N_WARM)

            # Matmul stream.  The PE queue is in-order, so the emission
            # order must match DMA arrival: while quarters still trickle in
            # (cold phase) go kc-major across the nh=0 groups; once all
            # data is on-chip, go group-sequential so groups complete at a
            # steady cadence and their epilogues (square -> per-subspace
            # sum -> k-max on Scalar/Vector) hide under later matmuls.
            zps = {}
            for bc in range(BC):
                for nh in range(2):
                    zps[(bc, nh)] = zpool.tile([128, 512], FP32,
                                               tag=f"zp_{bc}_{nh}",
                                               name=f"zp_{bc}_{nh}")

            def mm(bc, nh, i):
                nc.tensor.matmul(
                    zps[(bc, nh)][:],
                    x_sb[:, bc, 2 * i:2 * i + 2, :],
                    u_sb[nh][:, 2 * i:2 * i + 2, :],
                    start=(i == 0), stop=(i == 3),
                    perf_mode=DR, skip_group_check=True,
                )

            # per-group k-sums land in one staging tile; a single final
            # reduce_max over all 8 groups replaces 8 per-group maxes
            # (saves vector-queue time and gives the epilogue slack vs the
            # group-completion cadence)
            a_all = spool.tile([128, 2 * BC, K], FP32, tag="a_all")

            def epilogue(bc, nh, split):
                zp = zps[(bc, nh)]
                j = 2 * bc + nh
                e = epool.tile([128, 512], BF16, tag="e")
                e_v = e.rearrange("p (k c) -> p k c", c=d)
                # splitting the square/reduce pipelines Scalar against
                # Vector within one group's chain (used for the last group,
                # whose chain trails the matmul stream)
                nsplit = 4 if split else 1
                w = 512 // nsplit
                kw = K // nsplit
                for h in range(nsplit):
                    nc.scalar.square(e[:, h * w:(h + 1) * w],
                                     zp[:, h * w:(h + 1) * w])
                    nc.vector.reduce_sum(
                        a_all[:, j, h * kw:(h + 1) * kw],
                        e_v[:, h * kw:(h + 1) * kw, :],
                        axis=mybir.AxisListType.X,
                    )

            # pure group-sequential: group (0,0) cold-paces with u0's
            # quarter arrivals (fillers soak up the arrival gaps);
            # afterwards groups complete every ~0.9us so the epilogues
            # pipeline behind the stream
            for nh in range(2):
                for bc in range(BC):
                    for i in range(4):
                        mm(bc, nh, i)
                        if nh == 0 and bc == 0:
                            filler(4)
                        elif nh == 0 and bc == 1:
                            filler(1)
                    epilogue(bc, nh, split=(nh == 1 and bc == BC - 1))
                    if nh == 0 and bc == BC - 1:
                        filler(2)   # bridge into the u1-fed half-stream
            nc.vector.reduce_max(ostage[:, :], a_all[:],
                                 axis=mybir.AxisListType.X)
            nc.sync.dma_start(outp.ap()[:, :], ostage[:])

    nc.compile()
    return nc


def _prep(x, Us):
    x64 = x.astype(np.float64)
    xsq_sum = float(np.sum(x64 * x64))
    xt_f8 = np.ascontiguousarray(x.T).astype(ml_dtypes.float8_e4m3)    # (D, B)
    eye = np.eye(d)
    # fold chol(I - 0.5 U^T U) into U, all 64 subspaces at once
    Us64 = Us.astype(np.float64)
    G = np.einsum('skDa,skDb->skab', Us64, Us64)                        # (R,K,d,d)
    L = np.linalg.cholesky(eye[None, None] - 0.5 * G)
    Ut = np.einsum('skDa,skab->skDb', Us64, L)                          # (R,K,D,d)
    ut_all = Ut.transpose(2, 0, 1, 3).reshape(D, R * K * d)             # (D, 2048)
    ut_f8 = np.ascontiguousarray(ut_all * USCALE).astype(ml_dtypes.float8_e4m3)

    def onchip(arr):  # (D, cols) -> [128, KC*cols], partition-major
        cols = arr.shape[1]
        return np.ascontiguousarray(
            arr.reshape(KC, 128, cols).transpose(1, 0, 2).reshape(128, KC * cols))

    def onchip_x(arr):  # (D, NB) -> [128, BC*KC*128], batch-chunk-major
        return np.ascontiguousarray(
            arr.reshape(KC, 128, BC, 128)        # (kc, p, bc, c)
            .transpose(1, 2, 0, 3)               # (p, bc, kc, c)
            .reshape(128, BC * KC * 128))

    in_maps = []
    for c in range(NCORES):
        s2, b4 = c // 4, c % 4
        uts = ut_f8[:, 1024 * s2: 1024 * (s2 + 1)]
        in_maps.append({
            "xt": onchip_x(xt_f8[:, NB * b4: NB * (b4 + 1)]),
            "u0": onchip(uts[:, 0:512]),
            "u1": onchip(uts[:, 512:1024]),
        })
    return in_maps, xsq_sum


def kernel(x, Us, _trace=False):
    global LAST_RESULTS
    if "nc" not in _COMPILED:
        _COMPILED["nc"] = _build()
    nc = _COMPILED["nc"]
    in_maps, xsq_sum = _prep(np.asarray(x), np.asarray(Us))
    res = run_bass_kernel_spmd(nc, in_maps, core_ids=list(range(NCORES)),
                               trace=_trace)
    LAST_RESULTS = res
    base = 0.5 * xsq_sum / B
    inv = 1.0 / (USCALE * USCALE)
    obj = np.empty(R, np.float32)
    for r in range(R):
        s2, nh = r // 2, r % 2
        # outp[p, 2*bc+nh] = max_k (scaled) for sample bc*128+p, half nh
        vals = [res.results[4 * s2 + b]["outp"][:, nh::2] for b in range(4)]
        obj[r] = np.float32(base - inv * np.mean(
            [v.astype(np.float64).mean() for v in vals]))
    return obj
